# revision 40
# baseline (speedup 1.0000x reference)
"""Bidirectional column-chained GRU (vertical BiGRU over image columns) on 8 Trainium2 cores.

Topology: cores 0-3 run the forward GRU chain (batch quarters), cores 4-7 the
backward chain (rows pre-reversed on host). Each core runs the full C*S=16384
sequential GRU steps for its 8 batch rows in feature-major layout (128
partitions = hidden dim, free dim = batch). The C*S recurrence is inherently
serial, so 8 chains (4 batch groups x 2 directions) on 8 cores is the
latency-optimal partitioning; per-core work is one chain.

Math restructuring (validated vs reference in numpy):
  state hp1 = h + 1  (so n-path affine folds shrink the serial chain)
  tanh(x) = 2*sigmoid(2x) - 1  (single ACT table: sigmoid set, no switches)
  Per column c, for each gate g the rank-1 input contribution
  A_g,t = Wih_g*x_t + const_g is preloaded into PSUM with K=2 matmuls
  (const corrected by -Whh_g@1 for the hp1 shift; the const row of the rhs
  is a static ones row memset once in SBUF, so only x transfers); the
  recurrent matmul Whh_g @ hp1 then accumulates per step into PSUM slice t.
  Per step:
    r  = sigmoid(ps_r[t])                 ACT (PSUM src)
    u  = sigmoid(-ps_z[t])  (= 1-z)       ACT
    q  = r * ps_n[t]                      DVE
    w  = q + a_n[t]                       DVE
    v  = sigmoid(2w)                      ACT
    e1 = u * hp1; f = hp1 - e1            DVE
    e2 = 2*u*v                            DVE (scalar_tensor_tensor)
    hp1' = f + e2                         DVE
  The recurrent matmuls read the (f, e2) pair directly, so they need not
  wait for the final add; for the critical r/n gates the f-half issues as a
  separate matmul that executes while e2 is still on the DVE, hiding the
  weight load (split_mm). Recurrent weights and the (f, e2) pair are fp16:
  the PE streams fp16 4x faster than fp32, shortening the matmul hop of
  the serial chain. The steps run in a nested hardware loop (4 steps unrolled
  per iteration) which keeps the emitted program ~20x smaller than full
  unrolling - that matters because the axon path re-lowers and re-ships the
  program every call.
  Final per-column features h = hp1 - 1 are collected; the output head
  (fc + relu + softmax) runs on-device with a pairwise AllReduce between the
  fwd/bwd core of each batch group; each side contributes b_fc/2 via a K=1
  accumulate matmul so the sum carries the full bias.
  exp(relu(x)) == max(1, exp(x)); output is written fp16.

Host/runtime: the axon redirect re-creates a jax.jit per call, so a
persistent jax compilation cache is enabled to skip the per-call walrus
NEFF re-package (~0.5s host time per call otherwise).
"""

import numpy as np

import concourse.bass as bass
import concourse.bacc as bacc
import concourse.mybir as mybir
import concourse.tile as tile
from concourse.bass_utils import run_bass_kernel_spmd

B, S, C, H, O = 32, 128, 128, 128, 64
NCORES = 8
BL = B // 4          # batch rows per core (4 groups x 2 directions)
SB = S * BL          # rhs columns per image column
HS = SB // 2         # half-column psum width (one bank)
NSTEP = S // 2       # steps per half
f32 = mybir.dt.float32
f16 = mybir.dt.float16
FP = mybir.EngineType


def _emit(nc: bacc.Bacc, n_cols: int = C, loop_cols: int | None = None, skip_collective: bool = False, unroll: int = 4, reps: int = 1, split_mm: bool = True):
    AF = mybir.ActivationFunctionType
    OPM = mybir.AluOpType.mult
    import contextlib

    x_d = nc.dram_tensor("xcols", [n_cols, SB], f16, kind="ExternalInput").ap()
    hp10_d = nc.dram_tensor("hp10", [H, BL], f32, kind="ExternalInput").ap()
    whhrT_d = nc.dram_tensor("whhrT", [H, H], f16, kind="ExternalInput").ap()
    whhzT_d = nc.dram_tensor("whhzT", [H, H], f16, kind="ExternalInput").ap()
    whhnT_d = nc.dram_tensor("whhnT", [H, H], f16, kind="ExternalInput").ap()
    lcat_d = nc.dram_tensor("lcat", [2, 4 * H], f16, kind="ExternalInput").ap()
    wfcT_d = nc.dram_tensor("wfcT", [H, O], f32, kind="ExternalInput").ap()
    bfc_d = nc.dram_tensor("bfc_half", [1, O], f32, kind="ExternalInput").ap()
    out_d = nc.dram_tensor("out", [C * BL, O], f16, kind="ExternalOutput").ap()

    with tile.TileContext(nc) as tc:
        with tc.tile_pool(name="const", bufs=1) as cp:
            whhrT = cp.tile([H, H], f16)
            whhzT = cp.tile([H, H], f16)
            whhnT = cp.tile([H, H], f16)
            lcat = cp.tile([2, 4 * H], f16)
            wfcT = cp.tile([H, O], f32)
            bfc = cp.tile([1, O], f32)
            ones1 = cp.tile([1, H], f32)
            xo = cp.tile([2, SB], f16)
            hp1 = cp.tile([H, BL], f32)
            hall = cp.tile([H, C * BL], f32)
            r = cp.tile([H, BL], f32)
            u = cp.tile([H, BL], f32)
            q = cp.tile([H, BL], f32)
            w = cp.tile([H, BL], f32)
            v = cp.tile([H, BL], f32)
            e1 = cp.tile([H, BL], f32)
            fe2 = cp.tile([H, 2 * BL], f16)
            fp_, e2 = fe2[:, 0:BL], fe2[:, BL : 2 * BL]

            nc.sync.dma_start(whhrT[:], whhrT_d)
            nc.sync.dma_start(whhzT[:], whhzT_d)
            nc.sync.dma_start(whhnT[:], whhnT_d)
            nc.sync.dma_start(lcat[:], lcat_d)
            nc.sync.dma_start(wfcT[:], wfcT_d)
            nc.sync.dma_start(bfc[:], bfc_d)
            nc.vector.memset(ones1[:], 1.0)
            nc.vector.memset(xo[0:1, :], 1.0)
            nc.sync.dma_start(hp1[:], hp10_d)
            # (fp_, e2) must equal the state for the first step's matmuls
            nc.scalar.copy(fp_, hp1[:])
            nc.vector.memzero(e2)

            with (
                # reps>1 wraps the column loop for timing amplification only
                tc.For_i(0, reps, 1) if reps > 1 else contextlib.nullcontext(),
                tc.tile_pool(name="col", bufs=2) as colp,
                tc.tile_pool(name="ps", bufs=2, space="PSUM") as psp,
                tc.For_i(
                    0, n_cols if loop_cols is None else loop_cols, 1,
                    hint_engines=(FP.PE, FP.Activation, FP.DVE),
                ) as cv,
            ):
                nc.sync.dma_start(xo[1:2, :], x_d[bass.ds(cv, 1), :])

                def preload(half):
                    ps_r = psp.tile([H, HS], f32, tag="ps_r", name=f"ps_r{half}")
                    ps_z = psp.tile([H, HS], f32, tag="ps_z", name=f"ps_z{half}")
                    ps_n = psp.tile([H, HS], f32, tag="ps_n", name=f"ps_n{half}")
                    ps_t = psp.tile([H, HS], f32, tag="ps_t", name=f"ps_t{half}")
                    a_n = colp.tile([H, HS], f32, tag="a_n", name=f"a_n{half}")
                    xh = xo[:, half * HS : (half + 1) * HS]
                    # r first (gates the chain head), n+t next (needed by q/w
                    # at step 0), z last (the u path has slack)
                    nc.tensor.matmul(ps_r[:], lcat[:, 0:H], xh, start=True, stop=True)
                    nc.tensor.matmul(ps_n[:], lcat[:, 2 * H : 3 * H], xh, start=True, stop=True)
                    nc.tensor.matmul(ps_t[:], lcat[:, 3 * H : 4 * H], xh, start=True, stop=True)
                    nc.tensor.matmul(ps_z[:], lcat[:, H : 2 * H], xh, start=True, stop=True)
                    nc.scalar.copy(a_n[:], ps_t[:])
                    return ps_r, ps_z, ps_n, a_n

                def step1(ph, sl):
                    ps_r, ps_z, ps_n, a_n = ph
                    hp1v = fe2[:].rearrange("p (a o) -> p a o", a=2)
                    if split_mm:
                        # r/n accumulate fp_ early (ready before e2, hides the
                        # weight load behind the e2 DVE latency), e2 late
                        for ps, w_ in ((ps_r, whhrT), (ps_n, whhnT)):
                            nc.tensor.matmul(
                                ps[:, sl], w_[:], fp_, start=False, stop=False,
                                skip_group_check=True,
                            )
                            nc.tensor.matmul(
                                ps[:, sl], w_[:], e2, start=False, stop=True,
                                skip_group_check=True,
                            )
                        o_z = bass.broadcast_tensor_aps(
                            ps_z[:, sl].rearrange("p (a o) -> p a o", a=1), hp1v
                        )[0]
                        nc.tensor.matmul(
                            o_z, whhzT[:], hp1v, start=False, stop=True,
                            skip_group_check=True,
                        )
                    else:
                        outs = [
                            bass.broadcast_tensor_aps(
                                ps[:, sl].rearrange("p (a o) -> p a o", a=1), hp1v
                            )[0]
                            for ps in (ps_r, ps_n, ps_z)
                        ]
                        for o_, w_ in zip(outs, (whhrT, whhnT, whhzT)):
                            nc.tensor.matmul(
                                o_, w_[:], hp1v, start=False, stop=True,
                                skip_group_check=True,
                            )
                    nc.scalar.activation(r[:], ps_r[:, sl], AF.Sigmoid)
                    nc.scalar.activation(u[:], ps_z[:, sl], AF.Sigmoid, scale=-1.0)
                    nc.vector.tensor_mul(q[:], r[:], ps_n[:, sl])
                    nc.vector.tensor_add(w[:], q[:], a_n[:, sl])
                    nc.scalar.activation(v[:], w[:], AF.Sigmoid, scale=2.0)
                    nc.vector.tensor_mul(e1[:], u[:], hp1[:])
                    nc.vector.tensor_sub(fp_, hp1[:], e1[:])
                    nc.vector.scalar_tensor_tensor(
                        e2, u[:], 2.0, v[:], op0=OPM, op1=OPM
                    )
                    nc.vector.tensor_add(hp1[:], fp_, e2)

                def half_steps(ph):
                    if unroll > 1:
                        with tc.For_i(0, NSTEP, unroll) as ti:
                            for k in range(unroll):
                                step1(ph, bass.ds(ti * BL + k * BL, BL))
                    else:
                        with tc.For_i(0, NSTEP, 1) as ti:
                            step1(ph, bass.ds(ti * BL, BL))

                half_steps(preload(0))
                half_steps(preload(1))
                nc.vector.tensor_scalar_add(
                    hall[:, bass.ts(cv, BL)], hp1[:], -1.0
                )

            # output head: partial logits -> allreduce(fwd,bwd) -> softmax(relu(.))
            with (
                tc.tile_pool(name="fc", bufs=1) as fcp,
                tc.tile_pool(name="psfc", bufs=1, space="PSUM") as psfc,
                tc.tile_pool(name="dramp", bufs=1, space="DRAM") as dp,
            ):
                KB = (C * BL) // H  # 8 column blocks of 128
                lps = psfc.tile([128, KB * O], f32)
                for k in range(KB):
                    # logits block + b_fc/2 (summed to b_fc by the AllReduce)
                    nc.tensor.matmul(
                        lps[:, k * O : (k + 1) * O],
                        hall[:, k * H : (k + 1) * H],
                        wfcT[:],
                        start=True,
                        stop=False,
                    )
                    nc.tensor.matmul(
                        lps[:, k * O : (k + 1) * O],
                        ones1[:],
                        bfc[:],
                        start=False,
                        stop=True,
                    )
                # fp16 collective payload: logits are O(1), fp16 error ~1e-3
                lsb = fcp.tile([128, KB * O], f16)
                nc.scalar.copy(lsb[:], lps[:])
                lloc = dp.tile([C * BL, O], f16)
                lred = dp.tile([C * BL, O], f16)
                nc.sync.dma_start(
                    lloc.rearrange("(k p) o -> p k o", p=128),
                    lsb[:].rearrange("p (k o) -> p k o", k=KB),
                )
                if skip_collective:
                    nc.sync.dma_start(lred[:], lloc[:])
                else:
                    nc.gpsimd.collective_compute(
                        "AllReduce",
                        mybir.AluOpType.add,
                        replica_groups=[[0, 4], [1, 5], [2, 6], [3, 7]],
                        ins=[lloc.opt()],
                        outs=[lred.opt()],
                    )
                lsum = fcp.tile([128, KB * O], f16)
                nc.sync.dma_start(
                    lsum[:].rearrange("p (k o) -> p k o", k=KB),
                    lred.rearrange("(k p) o -> p k o", p=128),
                )
                ex = fcp.tile([128, KB * O], f32)
                nc.scalar.activation(ex[:], lsum[:], AF.Exp)
                # exp(relu(x)) == max(1, exp(x))
                nc.vector.tensor_scalar_max(ex[:], ex[:], 1.0)
                sums = fcp.tile([128, KB], f32)
                nc.vector.tensor_reduce(
                    sums[:],
                    ex[:].rearrange("p (k o) -> p k o", k=KB),
                    axis=mybir.AxisListType.X,
                    op=mybir.AluOpType.add,
                )
                rs = fcp.tile([128, KB], f32)
                nc.vector.reciprocal(rs[:], sums[:])
                osb = fcp.tile([128, KB * O], f16)
                for k in range(KB):
                    nc.vector.tensor_scalar_mul(
                        osb[:, k * O : (k + 1) * O],
                        ex[:, k * O : (k + 1) * O],
                        rs[:, k : k + 1],
                    )
                nc.sync.dma_start(
                    out_d.rearrange("(k p) o -> p k o", p=128),
                    osb[:].rearrange("p (k o) -> p k o", k=KB),
                )


_CACHE = {}


def _enable_jax_compile_cache():
    # The axon redirect re-creates a jax.jit per call, so without a
    # persistent cache every run pays a full walrus NEFF re-package
    # (~0.35s host time). The disk cache keys on the HLO (which embeds the
    # compressed BIR), so kernel edits invalidate it naturally. The cache
    # key also differs per process (axon client attributes), so the first
    # call of each process must re-write an entry; its warm compile takes
    # ~0.35s, hence a 0.1s write threshold (which still keeps most tiny
    # CPU-backend jits out of the cache).
    import jax

    jax.config.update("jax_compilation_cache_dir", "/tmp/bass_jax_cache")
    jax.config.update("jax_persistent_cache_min_compile_time_secs", 0.1)


def _build():
    if "nc" not in _CACHE:
        _enable_jax_compile_cache()
        nc = bacc.Bacc("TRN2", target_bir_lowering=False, debug=False, num_devices=NCORES)
        _emit(nc)
        nc.compile()
        _CACHE["nc"] = nc
    return _CACHE["nc"]


def _dir_shared(inputs, d):
    """Per-direction weight prep, shared by the 4 cores of that direction."""
    sfx = "f" if d == 0 else "b"
    Wih = inputs[f"Wih_{sfx}"][:, 0]
    Whh = inputs[f"Whh_{sfx}"]
    bih = inputs[f"bih_{sfx}"]
    bhh = inputs[f"bhh_{sfx}"]
    Wr, Wz, Wn = Whh[:H], Whh[H : 2 * H], Whh[2 * H :]
    # row 0 multiplies the static ones row of xo, row 1 the DMA'd x row
    lcat = np.zeros((2, 4 * H), np.float32)
    lcat[1, 0:H] = Wih[:H]
    lcat[0, 0:H] = bih[:H] + bhh[:H] - Wr.sum(1)
    lcat[1, H : 2 * H] = Wih[H : 2 * H]
    lcat[0, H : 2 * H] = bih[H : 2 * H] + bhh[H : 2 * H] - Wz.sum(1)
    lcat[0, 2 * H : 3 * H] = bhh[2 * H :] - Wn.sum(1)
    lcat[1, 3 * H : 4 * H] = Wih[2 * H :]
    lcat[0, 3 * H : 4 * H] = bih[2 * H :]
    wfc_half = inputs["W_fc"][:, :H] if d == 0 else inputs["W_fc"][:, H:]
    return {
        "whhrT": np.ascontiguousarray(Wr.T).astype(np.float16),
        "whhzT": np.ascontiguousarray(Wz.T).astype(np.float16),
        "whhnT": np.ascontiguousarray(Wn.T).astype(np.float16),
        "lcat": lcat.astype(np.float16),
        "wfcT": np.ascontiguousarray(wfc_half.T).astype(np.float32),
        "bfc_half": (0.5 * inputs["b_fc"])[None, :].astype(np.float32),
    }


def _in_maps(inputs):
    xf = np.transpose(inputs["x"], (2, 1, 0)).astype(np.float16)  # (C, S, B)
    xdir = (xf, xf[:, ::-1, :])
    shared = (_dir_shared(inputs, 0), _dir_shared(inputs, 1))
    maps = []
    for core in range(NCORES):
        d, g = (0, core) if core < 4 else (1, core - 4)
        bsl = slice(g * BL, (g + 1) * BL)
        maps.append({
            "xcols": np.ascontiguousarray(xdir[d][:, :, bsl]).reshape(C, SB),
            "hp10": np.ascontiguousarray(
                (inputs["h_prev"][d, bsl] + 1.0).T
            ).astype(np.float32),
            **shared[d],
        })
    return maps


def _assemble(results):
    out = np.empty((B, C, O), np.float32)
    for g in range(4):
        o = results[g]["out"].astype(np.float32).reshape(C, BL, O)
        out[g * BL : (g + 1) * BL] = np.transpose(o, (1, 0, 2))
    return out


def kernel(**inputs) -> np.ndarray:
    inputs = {k: np.asarray(v, dtype=np.float32) for k, v in inputs.items()}
    nc = _build()
    res = run_bass_kernel_spmd(nc, _in_maps(inputs), core_ids=list(range(NCORES)))
    return _assemble(res.results)


# revision 43
# speedup vs baseline: 1.1265x; 1.1265x over previous
"""Bidirectional column-chained GRU (vertical BiGRU over image columns) on 8 Trainium2 cores.

Topology: cores 0-3 run the forward GRU chain (batch quarters), cores 4-7 the
backward chain (rows pre-reversed on host). Each core runs the full C*S=16384
sequential GRU steps for its 8 batch rows in feature-major layout (128
partitions = hidden dim, free dim = batch). The C*S recurrence is inherently
serial, so 8 chains (4 batch groups x 2 directions) on 8 cores is the
latency-optimal partitioning; per-core work is one chain.

Math restructuring (validated vs reference in numpy):
  state hp1 = h + 1  (so n-path affine folds shrink the serial chain)
  tanh(x) = 2*sigmoid(2x) - 1  (single ACT table: sigmoid set, no switches)
  Per column c, for each gate g the rank-1 input contribution
  A_g,t = Wih_g*x_t + const_g is preloaded into PSUM with K=2 matmuls
  (const corrected by -Whh_g@1 for the hp1 shift; the const row of the rhs
  is a static ones row memset once in SBUF, so only x transfers); the
  recurrent matmul Whh_g @ hp1 then accumulates per step into PSUM slice t.
  Per step:
    r  = sigmoid(ps_r[t])                 ACT (PSUM src)
    u  = sigmoid(-ps_z[t])  (= 1-z)       ACT
    q  = r * ps_n[t]                      DVE
    w  = q + a_n[t]                       DVE
    v  = sigmoid(2w)                      ACT
    e1 = u * hp1; f = hp1 - e1            DVE
    e2 = 2*u*v                            DVE (scalar_tensor_tensor)
    hp1' = f + e2                         DVE
  The recurrent matmuls read the (f, e2) pair directly, so they need not
  wait for the final add; for the critical r/n gates the f-half issues as a
  separate matmul that executes while e2 is still on the DVE, hiding the
  weight load (split_mm). Recurrent weights and the (f, e2) pair are fp16:
  the PE streams fp16 4x faster than fp32, shortening the matmul hop of
  the serial chain. The steps run in a nested hardware loop (4 steps unrolled
  per iteration) which keeps the emitted program ~20x smaller than full
  unrolling - that matters because the axon path re-lowers and re-ships the
  program every call.
  Final per-column features h = hp1 - 1 are collected; the output head
  (fc + relu + softmax) runs on-device with a pairwise AllReduce between the
  fwd/bwd core of each batch group; each side contributes b_fc/2 via a K=1
  accumulate matmul so the sum carries the full bias.
  exp(relu(x)) == max(1, exp(x)); output is written fp16.

Host/runtime: the axon redirect re-creates a jax.jit per call, so a
persistent jax compilation cache is enabled to skip the per-call walrus
NEFF re-package (~0.5s host time per call otherwise).
"""

import numpy as np

import concourse.bass as bass
import concourse.bacc as bacc
import concourse.mybir as mybir
import concourse.tile as tile
from concourse.bass_utils import run_bass_kernel_spmd

B, S, C, H, O = 32, 128, 128, 128, 64
NCORES = 8
BL = B // 4          # batch rows per core (4 groups x 2 directions)
SB = S * BL          # rhs columns per image column
HS = SB // 2         # half-column psum width (one bank)
NSTEP = S // 2       # steps per half
f32 = mybir.dt.float32
f16 = mybir.dt.float16
FP = mybir.EngineType


def _emit(nc: bacc.Bacc, n_cols: int = C, loop_cols: int | None = None, skip_collective: bool = False, unroll: int = 4, reps: int = 1, split_mm: bool = True, warm_cc: bool = False, gps_off: bool = False):
    AF = mybir.ActivationFunctionType
    OPM = mybir.AluOpType.mult
    import contextlib

    x_d = nc.dram_tensor("xcols", [n_cols, SB], f16, kind="ExternalInput").ap()
    hp10_d = nc.dram_tensor("hp10", [H, BL], f32, kind="ExternalInput").ap()
    whhrT_d = nc.dram_tensor("whhrT", [H, H], f16, kind="ExternalInput").ap()
    whhzT_d = nc.dram_tensor("whhzT", [H, H], f16, kind="ExternalInput").ap()
    whhnT_d = nc.dram_tensor("whhnT", [H, H], f16, kind="ExternalInput").ap()
    lcat_d = nc.dram_tensor("lcat", [2, 4 * H], f16, kind="ExternalInput").ap()
    wfcT_d = nc.dram_tensor("wfcT", [H, O], f32, kind="ExternalInput").ap()
    bfc_d = nc.dram_tensor("bfc_half", [1, O], f32, kind="ExternalInput").ap()
    out_d = nc.dram_tensor("out", [C * BL, O], f16, kind="ExternalOutput").ap()

    with tile.TileContext(nc) as tc:
        with tc.tile_pool(name="const", bufs=1) as cp:
            whhrT = cp.tile([H, H], f16)
            whhzT = cp.tile([H, H], f16)
            whhnT = cp.tile([H, H], f16)
            lcat = cp.tile([2, 4 * H], f16)
            wfcT = cp.tile([H, O], f32)
            bfc = cp.tile([1, O], f32)
            ones1 = cp.tile([1, H], f32)
            xo = cp.tile([2, SB], f16)
            hp1 = cp.tile([H, BL], f32)
            hall = cp.tile([H, C * BL], f32)
            r = cp.tile([H, BL], f32)
            u = cp.tile([H, BL], f32)
            q = cp.tile([H, BL], f32)
            w = cp.tile([H, BL], f32)
            v = cp.tile([H, BL], f32)
            e1 = cp.tile([H, BL], f32)
            fe2 = cp.tile([H, 2 * BL], f16)
            fp_, e2 = fe2[:, 0:BL], fe2[:, BL : 2 * BL]

            nc.sync.dma_start(whhrT[:], whhrT_d)
            nc.sync.dma_start(whhzT[:], whhzT_d)
            nc.sync.dma_start(whhnT[:], whhnT_d)
            nc.sync.dma_start(lcat[:], lcat_d)
            nc.sync.dma_start(wfcT[:], wfcT_d)
            nc.sync.dma_start(bfc[:], bfc_d)
            nc.vector.memset(ones1[:], 1.0)
            nc.vector.memset(xo[0:1, :], 1.0)
            nc.sync.dma_start(hp1[:], hp10_d)
            # (fp_, e2) must equal the state for the first step's matmuls
            nc.scalar.copy(fp_, hp1[:])
            nc.vector.memzero(e2)

            if warm_cc and not skip_collective:
                # tiny dummy AllReduce issued before the recurrence: if the
                # collective cost is per-execution comm setup, it amortizes
                # here under the column loop instead of serializing at the end
                with tc.tile_pool(name="dwarm", bufs=1, space="DRAM") as dwp:
                    wa = dwp.tile([1, 8], f32)
                    wb = dwp.tile([1, 8], f32)
                    nc.sync.dma_start(wa[:], hp10_d[0:1, 0:8])
                    nc.gpsimd.collective_compute(
                        "AllReduce",
                        mybir.AluOpType.add,
                        replica_groups=[[0, 4], [1, 5], [2, 6], [3, 7]],
                        ins=[wa.opt()],
                        outs=[wb.opt()],
                    )

            with (
                # reps>1 wraps the column loop for timing amplification only
                tc.For_i(0, reps, 1) if reps > 1 else contextlib.nullcontext(),
                tc.tile_pool(name="col", bufs=2) as colp,
                tc.tile_pool(name="ps", bufs=2, space="PSUM") as psp,
                tc.For_i(
                    0, n_cols if loop_cols is None else loop_cols, 1,
                    hint_engines=(FP.PE, FP.Activation, FP.DVE),
                ) as cv,
            ):
                nc.sync.dma_start(xo[1:2, :], x_d[bass.ds(cv, 1), :])

                def preload(half):
                    ps_r = psp.tile([H, HS], f32, tag="ps_r", name=f"ps_r{half}")
                    ps_z = psp.tile([H, HS], f32, tag="ps_z", name=f"ps_z{half}")
                    ps_n = psp.tile([H, HS], f32, tag="ps_n", name=f"ps_n{half}")
                    ps_t = psp.tile([H, HS], f32, tag="ps_t", name=f"ps_t{half}")
                    a_n = colp.tile([H, HS], f32, tag="a_n", name=f"a_n{half}")
                    xh = xo[:, half * HS : (half + 1) * HS]
                    # r first (gates the chain head), n+t next (needed by q/w
                    # at step 0), z last (the u path has slack)
                    nc.tensor.matmul(ps_r[:], lcat[:, 0:H], xh, start=True, stop=True)
                    nc.tensor.matmul(ps_n[:], lcat[:, 2 * H : 3 * H], xh, start=True, stop=True)
                    nc.tensor.matmul(ps_t[:], lcat[:, 3 * H : 4 * H], xh, start=True, stop=True)
                    nc.tensor.matmul(ps_z[:], lcat[:, H : 2 * H], xh, start=True, stop=True)
                    nc.scalar.copy(a_n[:], ps_t[:])
                    return ps_r, ps_z, ps_n, a_n

                def step1(ph, sl):
                    ps_r, ps_z, ps_n, a_n = ph
                    hp1v = fe2[:].rearrange("p (a o) -> p a o", a=2)
                    if split_mm:
                        # r/n accumulate fp_ early (ready before e2, hides the
                        # weight load behind the e2 DVE latency), e2 late
                        for ps, w_ in ((ps_r, whhrT), (ps_n, whhnT)):
                            nc.tensor.matmul(
                                ps[:, sl], w_[:], fp_, start=False, stop=False,
                                skip_group_check=True,
                            )
                            nc.tensor.matmul(
                                ps[:, sl], w_[:], e2, start=False, stop=True,
                                skip_group_check=True,
                            )
                        o_z = bass.broadcast_tensor_aps(
                            ps_z[:, sl].rearrange("p (a o) -> p a o", a=1), hp1v
                        )[0]
                        nc.tensor.matmul(
                            o_z, whhzT[:], hp1v, start=False, stop=True,
                            skip_group_check=True,
                        )
                    else:
                        outs = [
                            bass.broadcast_tensor_aps(
                                ps[:, sl].rearrange("p (a o) -> p a o", a=1), hp1v
                            )[0]
                            for ps in (ps_r, ps_n, ps_z)
                        ]
                        for o_, w_ in zip(outs, (whhrT, whhnT, whhzT)):
                            nc.tensor.matmul(
                                o_, w_[:], hp1v, start=False, stop=True,
                                skip_group_check=True,
                            )
                    nc.scalar.activation(r[:], ps_r[:, sl], AF.Sigmoid)
                    nc.scalar.activation(u[:], ps_z[:, sl], AF.Sigmoid, scale=-1.0)
                    nc.vector.tensor_mul(q[:], r[:], ps_n[:, sl])
                    nc.vector.tensor_add(w[:], q[:], a_n[:, sl])
                    nc.scalar.activation(v[:], w[:], AF.Sigmoid, scale=2.0)
                    # e1/fp_/hp1' are off the mm critical path; gps_off moves
                    # them to the idle GpSimd engine so the DVE queue holds
                    # only the critical q/w/e2 ops
                    eng = nc.gpsimd if gps_off else nc.vector
                    eng.tensor_mul(e1[:], u[:], hp1[:])
                    eng.tensor_sub(fp_, hp1[:], e1[:])
                    nc.vector.scalar_tensor_tensor(
                        e2, u[:], 2.0, v[:], op0=OPM, op1=OPM
                    )
                    eng.tensor_add(hp1[:], fp_, e2)

                def half_steps(ph):
                    if unroll > 1:
                        with tc.For_i(0, NSTEP, unroll) as ti:
                            for k in range(unroll):
                                step1(ph, bass.ds(ti * BL + k * BL, BL))
                    else:
                        with tc.For_i(0, NSTEP, 1) as ti:
                            step1(ph, bass.ds(ti * BL, BL))

                half_steps(preload(0))
                half_steps(preload(1))
                nc.vector.tensor_scalar_add(
                    hall[:, bass.ts(cv, BL)], hp1[:], -1.0
                )

            # output head: partial logits -> allreduce(fwd,bwd) -> softmax(relu(.))
            with (
                tc.tile_pool(name="fc", bufs=1) as fcp,
                tc.tile_pool(name="psfc", bufs=1, space="PSUM") as psfc,
                tc.tile_pool(name="dramp", bufs=1, space="DRAM") as dp,
            ):
                KB = (C * BL) // H  # 8 column blocks of 128
                lps = psfc.tile([128, KB * O], f32)
                for k in range(KB):
                    # logits block + b_fc/2 (summed to b_fc by the AllReduce)
                    nc.tensor.matmul(
                        lps[:, k * O : (k + 1) * O],
                        hall[:, k * H : (k + 1) * H],
                        wfcT[:],
                        start=True,
                        stop=False,
                    )
                    nc.tensor.matmul(
                        lps[:, k * O : (k + 1) * O],
                        ones1[:],
                        bfc[:],
                        start=False,
                        stop=True,
                    )
                # fp16 collective payload: logits are O(1), fp16 error ~1e-3
                lsb = fcp.tile([128, KB * O], f16)
                nc.scalar.copy(lsb[:], lps[:])
                lloc = dp.tile([C * BL, O], f16)
                lred = dp.tile([C * BL, O], f16)
                nc.sync.dma_start(
                    lloc.rearrange("(k p) o -> p k o", p=128),
                    lsb[:].rearrange("p (k o) -> p k o", k=KB),
                )
                if skip_collective:
                    nc.sync.dma_start(lred[:], lloc[:])
                else:
                    nc.gpsimd.collective_compute(
                        "AllReduce",
                        mybir.AluOpType.add,
                        replica_groups=[[0, 4], [1, 5], [2, 6], [3, 7]],
                        ins=[lloc.opt()],
                        outs=[lred.opt()],
                    )
                lsum = fcp.tile([128, KB * O], f16)
                nc.sync.dma_start(
                    lsum[:].rearrange("p (k o) -> p k o", k=KB),
                    lred.rearrange("(k p) o -> p k o", p=128),
                )
                ex = fcp.tile([128, KB * O], f32)
                nc.scalar.activation(ex[:], lsum[:], AF.Exp)
                # exp(relu(x)) == max(1, exp(x))
                nc.vector.tensor_scalar_max(ex[:], ex[:], 1.0)
                sums = fcp.tile([128, KB], f32)
                nc.vector.tensor_reduce(
                    sums[:],
                    ex[:].rearrange("p (k o) -> p k o", k=KB),
                    axis=mybir.AxisListType.X,
                    op=mybir.AluOpType.add,
                )
                rs = fcp.tile([128, KB], f32)
                nc.vector.reciprocal(rs[:], sums[:])
                osb = fcp.tile([128, KB * O], f16)
                for k in range(KB):
                    nc.vector.tensor_scalar_mul(
                        osb[:, k * O : (k + 1) * O],
                        ex[:, k * O : (k + 1) * O],
                        rs[:, k : k + 1],
                    )
                nc.sync.dma_start(
                    out_d.rearrange("(k p) o -> p k o", p=128),
                    osb[:].rearrange("p (k o) -> p k o", k=KB),
                )


_CACHE = {}


def _enable_jax_compile_cache():
    # The axon redirect re-creates a jax.jit per call, so without a
    # persistent cache every run pays a full walrus NEFF re-package
    # (~0.35s host time). The disk cache keys on the HLO (which embeds the
    # compressed BIR), so kernel edits invalidate it naturally. The cache
    # key also differs per process (axon client attributes), so the first
    # call of each process must re-write an entry; its warm compile takes
    # ~0.35s, hence a 0.1s write threshold (which still keeps most tiny
    # CPU-backend jits out of the cache).
    import jax

    jax.config.update("jax_compilation_cache_dir", "/tmp/bass_jax_cache")
    jax.config.update("jax_persistent_cache_min_compile_time_secs", 0.1)


def _build():
    if "nc" not in _CACHE:
        _enable_jax_compile_cache()
        nc = bacc.Bacc("TRN2", target_bir_lowering=False, debug=False, num_devices=NCORES)
        _emit(nc)
        nc.compile()
        _CACHE["nc"] = nc
    return _CACHE["nc"]


def _dir_shared(inputs, d):
    """Per-direction weight prep, shared by the 4 cores of that direction."""
    sfx = "f" if d == 0 else "b"
    Wih = inputs[f"Wih_{sfx}"][:, 0]
    Whh = inputs[f"Whh_{sfx}"]
    bih = inputs[f"bih_{sfx}"]
    bhh = inputs[f"bhh_{sfx}"]
    Wr, Wz, Wn = Whh[:H], Whh[H : 2 * H], Whh[2 * H :]
    # row 0 multiplies the static ones row of xo, row 1 the DMA'd x row
    lcat = np.zeros((2, 4 * H), np.float32)
    lcat[1, 0:H] = Wih[:H]
    lcat[0, 0:H] = bih[:H] + bhh[:H] - Wr.sum(1)
    lcat[1, H : 2 * H] = Wih[H : 2 * H]
    lcat[0, H : 2 * H] = bih[H : 2 * H] + bhh[H : 2 * H] - Wz.sum(1)
    lcat[0, 2 * H : 3 * H] = bhh[2 * H :] - Wn.sum(1)
    lcat[1, 3 * H : 4 * H] = Wih[2 * H :]
    lcat[0, 3 * H : 4 * H] = bih[2 * H :]
    wfc_half = inputs["W_fc"][:, :H] if d == 0 else inputs["W_fc"][:, H:]
    return {
        "whhrT": np.ascontiguousarray(Wr.T).astype(np.float16),
        "whhzT": np.ascontiguousarray(Wz.T).astype(np.float16),
        "whhnT": np.ascontiguousarray(Wn.T).astype(np.float16),
        "lcat": lcat.astype(np.float16),
        "wfcT": np.ascontiguousarray(wfc_half.T).astype(np.float32),
        "bfc_half": (0.5 * inputs["b_fc"])[None, :].astype(np.float32),
    }


def _in_maps(inputs):
    xf = np.transpose(inputs["x"], (2, 1, 0)).astype(np.float16)  # (C, S, B)
    xdir = (xf, xf[:, ::-1, :])
    shared = (_dir_shared(inputs, 0), _dir_shared(inputs, 1))
    maps = []
    for core in range(NCORES):
        d, g = (0, core) if core < 4 else (1, core - 4)
        bsl = slice(g * BL, (g + 1) * BL)
        maps.append({
            "xcols": np.ascontiguousarray(xdir[d][:, :, bsl]).reshape(C, SB),
            "hp10": np.ascontiguousarray(
                (inputs["h_prev"][d, bsl] + 1.0).T
            ).astype(np.float32),
            **shared[d],
        })
    return maps


def _assemble(results):
    out = np.empty((B, C, O), np.float32)
    for g in range(4):
        o = results[g]["out"].astype(np.float32).reshape(C, BL, O)
        out[g * BL : (g + 1) * BL] = np.transpose(o, (1, 0, 2))
    return out


def kernel(**inputs) -> np.ndarray:
    inputs = {k: np.asarray(v, dtype=np.float32) for k, v in inputs.items()}
    nc = _build()
    res = run_bass_kernel_spmd(nc, _in_maps(inputs), core_ids=list(range(NCORES)))
    return _assemble(res.results)


# revision 45
# speedup vs baseline: 1.3445x; 1.1935x over previous
"""Bidirectional column-chained GRU (vertical BiGRU over image columns) on 8 Trainium2 cores.

Topology: cores 0-3 run the forward GRU chain (batch quarters), cores 4-7 the
backward chain (rows pre-reversed on host). Each core runs the full C*S=16384
sequential GRU steps for its 8 batch rows in feature-major layout (128
partitions = hidden dim, free dim = batch). The C*S recurrence is inherently
serial, so 8 chains (4 batch groups x 2 directions) on 8 cores is the
latency-optimal partitioning; per-core work is one chain.

Math restructuring (validated vs reference in numpy):
  state hp1 = h + 1  (so n-path affine folds shrink the serial chain)
  tanh(x) = 2*sigmoid(2x) - 1  (single ACT table: sigmoid set, no switches)
  Per column c, for each gate g the rank-1 input contribution
  A_g,t = Wih_g*x_t + const_g is preloaded into PSUM with K=2 matmuls
  (const corrected by -Whh_g@1 for the hp1 shift; the const row of the rhs
  is a static ones row memset once in SBUF, so only x transfers); the
  recurrent matmul Whh_g @ hp1 then accumulates per step into PSUM slice t.
  Per step:
    r  = sigmoid(ps_r[t])                 ACT (PSUM src)
    u  = sigmoid(-ps_z[t])  (= 1-z)       ACT
    q  = r * ps_n[t]                      DVE
    w  = q + a_n[t]                       DVE
    v  = sigmoid(2w)                      ACT
    e1 = u * hp1; f = hp1 - e1            DVE
    e2 = 2*u*v                            DVE (scalar_tensor_tensor)
    hp1' = f + e2                         DVE
  The recurrent matmuls read the (f, e2) pair directly, so they need not
  wait for the final add; for the critical r/n gates the f-half issues as a
  separate matmul that executes while e2 is still on the DVE, hiding the
  weight load (split_mm). Recurrent weights and the (f, e2) pair are fp16:
  the PE streams fp16 4x faster than fp32, shortening the matmul hop of
  the serial chain. The steps run in a nested hardware loop (4 steps unrolled
  per iteration) which keeps the emitted program ~20x smaller than full
  unrolling - that matters because the axon path re-lowers and re-ships the
  program every call.
  Final per-column features h = hp1 - 1 are collected; the output head
  (fc + relu + softmax) runs on-device with a pairwise AllReduce between the
  fwd/bwd core of each batch group; each side contributes b_fc/2 via a K=1
  accumulate matmul so the sum carries the full bias.
  exp(relu(x)) == max(1, exp(x)); output is written fp16.

Host/runtime: the axon redirect re-creates a jax.jit per call, so a
persistent jax compilation cache is enabled to skip the per-call walrus
NEFF re-package (~0.5s host time per call otherwise).
"""

import numpy as np

import concourse.bass as bass
import concourse.bacc as bacc
import concourse.mybir as mybir
import concourse.tile as tile
from concourse.bass_utils import run_bass_kernel_spmd

B, S, C, H, O = 32, 128, 128, 128, 64
NCORES = 8
BL = B // 4          # batch rows per core (4 groups x 2 directions)
SB = S * BL          # rhs columns per image column
HS = SB // 2         # half-column psum width (one bank)
NSTEP = S // 2       # steps per half
f32 = mybir.dt.float32
f16 = mybir.dt.float16
FP = mybir.EngineType


def _emit(nc: bacc.Bacc, n_cols: int = C, loop_cols: int | None = None, skip_collective: bool = False, unroll: int = 4, reps: int = 1, split_mm: bool = True, warm_cc: bool = False, gps_off: bool = False, stag: bool = False):
    AF = mybir.ActivationFunctionType
    OPM = mybir.AluOpType.mult
    import contextlib

    x_d = nc.dram_tensor("xcols", [n_cols, SB], f16, kind="ExternalInput").ap()
    hp10_d = nc.dram_tensor("hp10", [H, BL], f32, kind="ExternalInput").ap()
    whhrT_d = nc.dram_tensor("whhrT", [H, H], f16, kind="ExternalInput").ap()
    whhzT_d = nc.dram_tensor("whhzT", [H, H], f16, kind="ExternalInput").ap()
    whhnT_d = nc.dram_tensor("whhnT", [H, H], f16, kind="ExternalInput").ap()
    lcat_d = nc.dram_tensor("lcat", [2, 4 * H], f16, kind="ExternalInput").ap()
    wfcT_d = nc.dram_tensor("wfcT", [H, O], f32, kind="ExternalInput").ap()
    bfc_d = nc.dram_tensor("bfc_half", [1, O], f32, kind="ExternalInput").ap()
    out_d = nc.dram_tensor("out", [C * BL, O], f16, kind="ExternalOutput").ap()

    with tile.TileContext(nc) as tc:
        with tc.tile_pool(name="const", bufs=1) as cp:
            whhrT = cp.tile([H, H], f16)
            whhzT = cp.tile([H, H], f16)
            whhnT = cp.tile([H, H], f16)
            lcat = cp.tile([2, 4 * H], f16)
            wfcT = cp.tile([H, O], f32)
            bfc = cp.tile([1, O], f32)
            ones1 = cp.tile([1, H], f32)
            xo = cp.tile([2, SB], f16)
            hp1 = cp.tile([H, BL], f32)
            hall = cp.tile([H, C * BL], f32)
            r = cp.tile([H, BL], f32)
            u = cp.tile([H, BL], f32)
            q = cp.tile([H, BL], f32)
            w = cp.tile([H, BL], f32)
            v = cp.tile([H, BL], f32)
            e1 = cp.tile([H, BL], f32)
            fe2 = cp.tile([H, 2 * BL], f16)
            fp_, e2 = fe2[:, 0:BL], fe2[:, BL : 2 * BL]

            nc.sync.dma_start(whhrT[:], whhrT_d)
            nc.sync.dma_start(whhzT[:], whhzT_d)
            nc.sync.dma_start(whhnT[:], whhnT_d)
            nc.sync.dma_start(lcat[:], lcat_d)
            nc.sync.dma_start(wfcT[:], wfcT_d)
            nc.sync.dma_start(bfc[:], bfc_d)
            nc.vector.memset(ones1[:], 1.0)
            nc.vector.memset(xo[0:1, :], 1.0)
            nc.sync.dma_start(hp1[:], hp10_d)
            # (fp_, e2) must equal the state for the first step's matmuls
            nc.scalar.copy(fp_, hp1[:])
            nc.vector.memzero(e2)

            if warm_cc and not skip_collective:
                # tiny dummy AllReduce issued before the recurrence: if the
                # collective cost is per-execution comm setup, it amortizes
                # here under the column loop instead of serializing at the end
                with tc.tile_pool(name="dwarm", bufs=1, space="DRAM") as dwp:
                    wa = dwp.tile([1, 8], f32)
                    wb = dwp.tile([1, 8], f32)
                    nc.sync.dma_start(wa[:], hp10_d[0:1, 0:8])
                    nc.gpsimd.collective_compute(
                        "AllReduce",
                        mybir.AluOpType.add,
                        replica_groups=[[0, 4], [1, 5], [2, 6], [3, 7]],
                        ins=[wa.opt()],
                        outs=[wb.opt()],
                    )

            with (
                # reps>1 wraps the column loop for timing amplification only
                tc.For_i(0, reps, 1) if reps > 1 else contextlib.nullcontext(),
                tc.tile_pool(name="col", bufs=2) as colp,
                tc.tile_pool(name="ps", bufs=2, space="PSUM") as psp,
                tc.For_i(
                    0, n_cols if loop_cols is None else loop_cols, 1,
                    hint_engines=(FP.PE, FP.Activation, FP.DVE),
                ) as cv,
            ):
                nc.sync.dma_start(xo[1:2, :], x_d[bass.ds(cv, 1), :])

                def preload(half):
                    ps_r = psp.tile([H, HS], f32, tag="ps_r", name=f"ps_r{half}")
                    ps_z = psp.tile([H, HS], f32, tag="ps_z", name=f"ps_z{half}")
                    ps_n = psp.tile([H, HS], f32, tag="ps_n", name=f"ps_n{half}")
                    ps_t = psp.tile([H, HS], f32, tag="ps_t", name=f"ps_t{half}")
                    a_n = colp.tile([H, HS], f32, tag="a_n", name=f"a_n{half}")
                    xh = xo[:, half * HS : (half + 1) * HS]
                    # r first (gates the chain head), n+t next (needed by q/w
                    # at step 0), z last (the u path has slack)
                    nc.tensor.matmul(ps_r[:], lcat[:, 0:H], xh, start=True, stop=True)
                    nc.tensor.matmul(ps_n[:], lcat[:, 2 * H : 3 * H], xh, start=True, stop=True)
                    nc.tensor.matmul(ps_t[:], lcat[:, 3 * H : 4 * H], xh, start=True, stop=True)
                    nc.tensor.matmul(ps_z[:], lcat[:, H : 2 * H], xh, start=True, stop=True)
                    nc.scalar.copy(a_n[:], ps_t[:])
                    return ps_r, ps_z, ps_n, a_n

                def step1(ph, sl):
                    ps_r, ps_z, ps_n, a_n = ph
                    hp1v = fe2[:].rearrange("p (a o) -> p a o", a=2)
                    if split_mm:
                        # r/n accumulate fp_ early (ready before e2, hides the
                        # weight load behind the e2 DVE latency), e2 late
                        for ps, w_ in ((ps_r, whhrT), (ps_n, whhnT)):
                            nc.tensor.matmul(
                                ps[:, sl], w_[:], fp_, start=False, stop=False,
                                skip_group_check=True,
                            )
                            nc.tensor.matmul(
                                ps[:, sl], w_[:], e2, start=False, stop=True,
                                skip_group_check=True,
                            )
                        o_z = bass.broadcast_tensor_aps(
                            ps_z[:, sl].rearrange("p (a o) -> p a o", a=1), hp1v
                        )[0]
                        nc.tensor.matmul(
                            o_z, whhzT[:], hp1v, start=False, stop=True,
                            skip_group_check=True,
                        )
                    else:
                        outs = [
                            bass.broadcast_tensor_aps(
                                ps[:, sl].rearrange("p (a o) -> p a o", a=1), hp1v
                            )[0]
                            for ps in (ps_r, ps_n, ps_z)
                        ]
                        for o_, w_ in zip(outs, (whhrT, whhnT, whhzT)):
                            nc.tensor.matmul(
                                o_, w_[:], hp1v, start=False, stop=True,
                                skip_group_check=True,
                            )
                    nc.scalar.activation(r[:], ps_r[:, sl], AF.Sigmoid)
                    nc.scalar.activation(u[:], ps_z[:, sl], AF.Sigmoid, scale=-1.0)
                    nc.vector.tensor_mul(q[:], r[:], ps_n[:, sl])
                    nc.vector.tensor_add(w[:], q[:], a_n[:, sl])
                    nc.scalar.activation(v[:], w[:], AF.Sigmoid, scale=2.0)
                    # e1/fp_/hp1' are off the mm critical path; gps_off moves
                    # them to the idle GpSimd engine so the DVE queue holds
                    # only the critical q/w/e2 ops
                    eng = nc.gpsimd if gps_off else nc.vector
                    eng.tensor_mul(e1[:], u[:], hp1[:])
                    eng.tensor_sub(fp_, hp1[:], e1[:])
                    nc.vector.scalar_tensor_tensor(
                        e2, u[:], 2.0, v[:], op0=OPM, op1=OPM
                    )
                    eng.tensor_add(hp1[:], fp_, e2)

                def half_steps(ph):
                    # stag: staggered semaphore resets drop the per-iteration
                    # all-engine barrier from the inner loop's back edge
                    if unroll > 1:
                        with tc.For_i(0, NSTEP, unroll, staggered_reset=stag) as ti:
                            for k in range(unroll):
                                step1(ph, bass.ds(ti * BL + k * BL, BL))
                    else:
                        with tc.For_i(0, NSTEP, 1, staggered_reset=stag) as ti:
                            step1(ph, bass.ds(ti * BL, BL))

                half_steps(preload(0))
                half_steps(preload(1))
                nc.vector.tensor_scalar_add(
                    hall[:, bass.ts(cv, BL)], hp1[:], -1.0
                )

            # output head: partial logits -> allreduce(fwd,bwd) -> softmax(relu(.))
            with (
                tc.tile_pool(name="fc", bufs=1) as fcp,
                tc.tile_pool(name="psfc", bufs=1, space="PSUM") as psfc,
                tc.tile_pool(name="dramp", bufs=1, space="DRAM") as dp,
            ):
                KB = (C * BL) // H  # 8 column blocks of 128
                lps = psfc.tile([128, KB * O], f32)
                for k in range(KB):
                    # logits block + b_fc/2 (summed to b_fc by the AllReduce)
                    nc.tensor.matmul(
                        lps[:, k * O : (k + 1) * O],
                        hall[:, k * H : (k + 1) * H],
                        wfcT[:],
                        start=True,
                        stop=False,
                    )
                    nc.tensor.matmul(
                        lps[:, k * O : (k + 1) * O],
                        ones1[:],
                        bfc[:],
                        start=False,
                        stop=True,
                    )
                # fp16 collective payload: logits are O(1), fp16 error ~1e-3
                lsb = fcp.tile([128, KB * O], f16)
                nc.scalar.copy(lsb[:], lps[:])
                lloc = dp.tile([C * BL, O], f16)
                lred = dp.tile([C * BL, O], f16)
                nc.sync.dma_start(
                    lloc.rearrange("(k p) o -> p k o", p=128),
                    lsb[:].rearrange("p (k o) -> p k o", k=KB),
                )
                if skip_collective:
                    nc.sync.dma_start(lred[:], lloc[:])
                else:
                    nc.gpsimd.collective_compute(
                        "AllReduce",
                        mybir.AluOpType.add,
                        replica_groups=[[0, 4], [1, 5], [2, 6], [3, 7]],
                        ins=[lloc.opt()],
                        outs=[lred.opt()],
                    )
                lsum = fcp.tile([128, KB * O], f16)
                nc.sync.dma_start(
                    lsum[:].rearrange("p (k o) -> p k o", k=KB),
                    lred.rearrange("(k p) o -> p k o", p=128),
                )
                ex = fcp.tile([128, KB * O], f32)
                nc.scalar.activation(ex[:], lsum[:], AF.Exp)
                # exp(relu(x)) == max(1, exp(x))
                nc.vector.tensor_scalar_max(ex[:], ex[:], 1.0)
                sums = fcp.tile([128, KB], f32)
                nc.vector.tensor_reduce(
                    sums[:],
                    ex[:].rearrange("p (k o) -> p k o", k=KB),
                    axis=mybir.AxisListType.X,
                    op=mybir.AluOpType.add,
                )
                rs = fcp.tile([128, KB], f32)
                nc.vector.reciprocal(rs[:], sums[:])
                osb = fcp.tile([128, KB * O], f16)
                for k in range(KB):
                    nc.vector.tensor_scalar_mul(
                        osb[:, k * O : (k + 1) * O],
                        ex[:, k * O : (k + 1) * O],
                        rs[:, k : k + 1],
                    )
                nc.sync.dma_start(
                    out_d.rearrange("(k p) o -> p k o", p=128),
                    osb[:].rearrange("p (k o) -> p k o", k=KB),
                )


_CACHE = {}


def _enable_jax_compile_cache():
    # The axon redirect re-creates a jax.jit per call, so without a
    # persistent cache every run pays a full walrus NEFF re-package
    # (~0.35s host time). The disk cache keys on the HLO (which embeds the
    # compressed BIR), so kernel edits invalidate it naturally. The cache
    # key also differs per process (axon client attributes), so the first
    # call of each process must re-write an entry; its warm compile takes
    # ~0.35s, hence a 0.1s write threshold (which still keeps most tiny
    # CPU-backend jits out of the cache).
    import jax

    jax.config.update("jax_compilation_cache_dir", "/tmp/bass_jax_cache")
    jax.config.update("jax_persistent_cache_min_compile_time_secs", 0.1)


def _build():
    if "nc" not in _CACHE:
        _enable_jax_compile_cache()
        nc = bacc.Bacc("TRN2", target_bir_lowering=False, debug=False, num_devices=NCORES)
        _emit(nc)
        nc.compile()
        _CACHE["nc"] = nc
    return _CACHE["nc"]


def _dir_shared(inputs, d):
    """Per-direction weight prep, shared by the 4 cores of that direction."""
    sfx = "f" if d == 0 else "b"
    Wih = inputs[f"Wih_{sfx}"][:, 0]
    Whh = inputs[f"Whh_{sfx}"]
    bih = inputs[f"bih_{sfx}"]
    bhh = inputs[f"bhh_{sfx}"]
    Wr, Wz, Wn = Whh[:H], Whh[H : 2 * H], Whh[2 * H :]
    # row 0 multiplies the static ones row of xo, row 1 the DMA'd x row
    lcat = np.zeros((2, 4 * H), np.float32)
    lcat[1, 0:H] = Wih[:H]
    lcat[0, 0:H] = bih[:H] + bhh[:H] - Wr.sum(1)
    lcat[1, H : 2 * H] = Wih[H : 2 * H]
    lcat[0, H : 2 * H] = bih[H : 2 * H] + bhh[H : 2 * H] - Wz.sum(1)
    lcat[0, 2 * H : 3 * H] = bhh[2 * H :] - Wn.sum(1)
    lcat[1, 3 * H : 4 * H] = Wih[2 * H :]
    lcat[0, 3 * H : 4 * H] = bih[2 * H :]
    wfc_half = inputs["W_fc"][:, :H] if d == 0 else inputs["W_fc"][:, H:]
    return {
        "whhrT": np.ascontiguousarray(Wr.T).astype(np.float16),
        "whhzT": np.ascontiguousarray(Wz.T).astype(np.float16),
        "whhnT": np.ascontiguousarray(Wn.T).astype(np.float16),
        "lcat": lcat.astype(np.float16),
        "wfcT": np.ascontiguousarray(wfc_half.T).astype(np.float32),
        "bfc_half": (0.5 * inputs["b_fc"])[None, :].astype(np.float32),
    }


def _in_maps(inputs):
    xf = np.transpose(inputs["x"], (2, 1, 0)).astype(np.float16)  # (C, S, B)
    xdir = (xf, xf[:, ::-1, :])
    shared = (_dir_shared(inputs, 0), _dir_shared(inputs, 1))
    maps = []
    for core in range(NCORES):
        d, g = (0, core) if core < 4 else (1, core - 4)
        bsl = slice(g * BL, (g + 1) * BL)
        maps.append({
            "xcols": np.ascontiguousarray(xdir[d][:, :, bsl]).reshape(C, SB),
            "hp10": np.ascontiguousarray(
                (inputs["h_prev"][d, bsl] + 1.0).T
            ).astype(np.float32),
            **shared[d],
        })
    return maps


def _assemble(results):
    out = np.empty((B, C, O), np.float32)
    for g in range(4):
        o = results[g]["out"].astype(np.float32).reshape(C, BL, O)
        out[g * BL : (g + 1) * BL] = np.transpose(o, (1, 0, 2))
    return out


def kernel(**inputs) -> np.ndarray:
    inputs = {k: np.asarray(v, dtype=np.float32) for k, v in inputs.items()}
    nc = _build()
    res = run_bass_kernel_spmd(nc, _in_maps(inputs), core_ids=list(range(NCORES)))
    return _assemble(res.results)
